# revision 3
# baseline (speedup 1.0000x reference)
"""Trainium2 Bass kernel for nn_CrossAttention_70866960384676.

Reference semantics: cross-attention where only token 0 of each batch is the
query; K/V projections span the full sequence; rotary uses head-index
positions (constant over sequence).

Algebraic reduction (validated vs reference at ~1e-6 rel in fp32):
  q_rot = rotary(x0 @ Wq);  e = rotary_adjoint(q_rot) * DH^-0.5
  U[:, h] = Wk[:, h*DH:(h+1)*DH] @ e[h]          (per batch; 1024x16)
  logits = x @ U                                  (N x H)
  a = exp(logits);  S = sum_n a
  ynorm = (a.T @ x) / S                           (H x 1024)
  z[h*DH:(h+1)*DH] = ynorm[h] @ Wv[:, h*DH:(h+1)*DH]
  out = z @ Wp + bp

This turns a 275-GFLOP dense problem into a DMA-bound streaming problem
(~50 MB/core).  Sharding: pure data-parallel, 2 batches per core, 8 cores.

On-chip structure per core (hot matmuls in float32r; transposes in fp32):
  pass-1 needs x with dim on partitions -> PE transposes of each x tile
  pass-2 consumes x in natural layout (f32r-rounded)
"""
import numpy as np
from contextlib import ExitStack

import concourse.bass as bass
import concourse.tile as tile
from concourse import bacc, mybir
from concourse.bass import ds
from concourse.bass_utils import run_bass_kernel_spmd
from concourse.masks import make_identity

dt = mybir.dt
F32 = dt.float32
F32R = dt.float32r
AF = mybir.ActivationFunctionType

B, N, DIM, H, DH = 16, 4096, 1024, 16, 64
NCORES = 8
BPC = B // NCORES          # batches per core
NCH = DIM // 128           # 8 dim chunks
TOK = 256                  # tokens per main-loop tile
NT = N // TOK              # 16 tiles per batch
THETA = 10000.0
SCALE = DH ** -0.5

_CACHE = {}


# ---------------------------------------------------------------- host tables
def _host_tables():
    inv = 1.0 / (THETA ** (np.arange(0, DH, 2, dtype=np.float64) / DH))
    t = np.arange(H, dtype=np.float64)
    fr = t[:, None] * inv[None, :]
    emb = np.concatenate([fr, fr], -1)                      # (H, DH)
    cos = np.cos(emb).reshape(1, DIM)
    sin = np.sin(emb).reshape(1, DIM)
    cosq = np.broadcast_to(cos, (BPC, DIM)).astype(np.float32).copy()
    sinq = np.broadcast_to(sin, (BPC, DIM)).astype(np.float32).copy()
    cose = (cosq * SCALE).astype(np.float32)
    sine = (sinq * SCALE).astype(np.float32)
    mask = np.zeros((H, DIM), np.float32)                   # head-block mask
    for h in range(H):
        mask[h, h * DH:(h + 1) * DH] = 1.0
    return cosq, sinq, cose, sine, mask


# ------------------------------------------------------------------ bass emit
def _emit(tc, T):
    nc = tc.nc
    with ExitStack() as ctx:
        persist = ctx.enter_context(tc.tile_pool(name="persist", bufs=1))
        ptmp = ctx.enter_context(tc.tile_pool(name="ptmp", bufs=2))
        wtmp = ctx.enter_context(tc.tile_pool(name="wtmp", bufs=2))
        xin_p = ctx.enter_context(tc.tile_pool(name="xin", bufs=2))
        xr_p = ctx.enter_context(tc.tile_pool(name="xr", bufs=3))
        xt_p = ctx.enter_context(tc.tile_pool(name="xt", bufs=2))
        at_p = ctx.enter_context(tc.tile_pool(name="at", bufs=2))
        asb_p = ctx.enter_context(tc.tile_pool(name="asb", bufs=2))
        ps_stage = ctx.enter_context(
            tc.tile_pool(name="ps_stage", bufs=2, space="PSUM"))
        ps_acc = ctx.enter_context(
            tc.tile_pool(name="ps_acc", bufs=2, space="PSUM"))
        ps_a = ctx.enter_context(tc.tile_pool(name="ps_a", bufs=2, space="PSUM"))
        ps_y = ctx.enter_context(tc.tile_pool(name="ps_y", bufs=1, space="PSUM"))

        # ---------------- constants ----------------
        ident = persist.tile([128, 128], F32)
        make_identity(nc, ident)
        cosq = persist.tile([BPC, DIM], F32)
        sinq = persist.tile([BPC, DIM], F32)
        cose = persist.tile([BPC, DIM], F32)
        sine = persist.tile([BPC, DIM], F32)
        mask = persist.tile([H, DIM], F32)
        bp_sb = persist.tile([1, DIM], F32)
        x0t_sb = persist.tile([128, NCH, BPC], F32)
        for name, t_ in [("cosq", cosq), ("sinq", sinq), ("cose", cose),
                         ("sine", sine), ("mask", mask), ("bp", bp_sb),
                         ("x0t", x0t_sb)]:
            nc.sync.dma_start(t_[:], T[name])

        ones16f = persist.tile([H, 1], F32)
        nc.vector.memset(ones16f[:], 1.0)
        ones16 = persist.tile([H, 1], F32R)
        nc.vector.tensor_copy(ones16[:], ones16f[:])

        x0r = persist.tile([128, NCH, BPC], F32R)
        nc.vector.tensor_copy(x0r[:], x0t_sb[:])

        # ---------------- prologue: weights Wq / WkT ----------------
        def load_weight_rounded(dram_ap, pool, tag):
            w = pool.tile([128, NCH, DIM], F32R, tag=tag)
            for ch in range(NCH):
                tmp = wtmp.tile([128, DIM], F32, tag="wtmp")
                nc.sync.dma_start(tmp[:], dram_ap[ch * 128:(ch + 1) * 128, :])
                nc.vector.tensor_copy(w[:, ch, :], tmp[:])
            return w

        with tc.tile_pool(name="w1", bufs=1) as w1, \
                tc.tile_pool(name="rot", bufs=3) as rot:
            wqr = load_weight_rounded(T["wq"], w1, "wbig")

            # q = x0 @ Wq  -> psum (BPC, 1024) in two halves
            qh = [ps_acc.tile([BPC, 512], F32, tag="acc", name=f"qh{_}") for _ in range(2)]
            for hf in range(2):
                for ch in range(NCH):
                    nc.tensor.matmul(qh[hf][:], x0r[:, ch, :],
                                     wqr[:, ch, ds(hf * 512, 512)],
                                     start=(ch == 0), stop=(ch == NCH - 1))
            q_sb = rot.tile([BPC, DIM], F32, tag="rot")
            for hf in range(2):
                nc.scalar.copy(out=q_sb[:, ds(hf * 512, 512)], in_=qh[hf][:])

            # rotary on q (view (BPC, H, DH))
            def halves(t_):
                v = t_[:].rearrange("p (h d) -> p h d", h=H)
                return v[:, :, 0:DH // 2], v[:, :, DH // 2:DH]

            rh = rot.tile([BPC, DIM], F32, tag="rot")
            rh_lo, rh_hi = halves(rh)
            q_lo, q_hi = halves(q_sb)
            nc.vector.tensor_scalar_mul(rh_lo, q_hi, -1.0)
            nc.vector.tensor_copy(rh_hi, q_lo)
            t1 = rot.tile([BPC, DIM], F32, tag="rot")
            nc.vector.tensor_mul(t1[:], q_sb[:], cosq[:])
            t2 = rot.tile([BPC, DIM], F32, tag="rot")
            nc.vector.tensor_mul(t2[:], rh[:], sinq[:])
            qrot = rot.tile([BPC, DIM], F32, tag="rot")
            nc.vector.tensor_add(qrot[:], t1[:], t2[:])
            # e = qrot*cose + rotadj(qrot*sine)   (cose/sine carry the 1/8)
            y2 = rot.tile([BPC, DIM], F32, tag="rot")
            nc.vector.tensor_mul(y2[:], qrot[:], sine[:])
            radj = rot.tile([BPC, DIM], F32, tag="rot")
            ra_lo, ra_hi = halves(radj)
            y2_lo, y2_hi = halves(y2)
            nc.vector.tensor_copy(ra_lo, y2_hi)
            nc.vector.tensor_scalar_mul(ra_hi, y2_lo, -1.0)
            t3 = rot.tile([BPC, DIM], F32, tag="rot")
            nc.vector.tensor_mul(t3[:], qrot[:], cose[:])
            e_sb = rot.tile([BPC, DIM], F32, tag="rot")
            nc.vector.tensor_add(e_sb[:], t3[:], radj[:])

            # e -> eT (128, NCH, BPC)
            etp = ps_stage.tile([128, NCH, BPC], F32, tag="stage")
            for ch in range(NCH):
                nc.tensor.transpose(etp[:, ch, :], e_sb[:, ds(ch * 128, 128)],
                                    ident[0:BPC, 0:BPC])
            eT = persist.tile([128, NCH, BPC], F32)
            nc.vector.tensor_copy(eT[:], etp[:])

            # E_b block-diagonal (128, NCH, H), then U_b = (E_b.T @ WkT).T
            wktr = load_weight_rounded(T["wkt"], w1, "wbig")
            U = []
            for b in range(BPC):
                ef = persist.tile([128, NCH, H], F32, tag=f"ef{b}")
                nc.vector.memset(ef[:], 0.0)
                eflat = ef[:].rearrange("p a b -> p (a b)")
                nc.vector.tensor_copy(eflat[0:64, 0:127:18], eT[0:64, :, b])
                nc.vector.tensor_copy(eflat[64:128, 1:128:18], eT[64:128, :, b])
                er = persist.tile([128, NCH, H], F32R, tag=f"er{b}")
                nc.vector.tensor_copy(er[:], ef[:])

                uth = [ps_acc.tile([H, 512], F32, tag="acc", name=f"uth{_}") for _ in range(2)]
                for hf in range(2):
                    for ch in range(NCH):
                        nc.tensor.matmul(uth[hf][:], er[:, ch, :],
                                         wktr[:, ch, ds(hf * 512, 512)],
                                         start=(ch == 0), stop=(ch == NCH - 1))
                utr = ptmp.tile([H, DIM], F32, tag="utr", bufs=1)
                for hf in range(2):
                    nc.scalar.copy(out=utr[:, ds(hf * 512, 512)], in_=uth[hf][:])
                ustage = ps_stage.tile([128, NCH, H], F32, tag="stage")
                for ch in range(NCH):
                    nc.tensor.transpose(ustage[:, ch, :],
                                        utr[:, ds(ch * 128, 128)],
                                        ident[0:H, 0:H])
                u_b = persist.tile([128, NCH, H], F32R, tag=f"u{b}")
                nc.vector.tensor_copy(u_b[:], ustage[:])
                U.append(u_b)

        # ---------------- main loop ----------------
        Spart = [persist.tile([H, NT], F32, tag=f"sp{b}", name=f"sp{b}") for b in range(BPC)]
        ynorm = [persist.tile([H, DIM], F32, tag=f"yn{b}", name=f"yn{b}") for b in range(BPC)]

        with tc.tile_pool(name="w2", bufs=1) as w2:
            wvr = load_weight_rounded(T["wv"], w2, "wvr")
            wpr = load_weight_rounded(T["wp"], w2, "wpr")

            for b in range(BPC):
                yps = ps_y.tile([H, 2, 512], F32, tag="y")
                for i in range(NT):
                    t0 = i * TOK
                    xt_f32 = xin_p.tile([128, 2, DIM], F32, tag="xin")
                    src = T["x"][b, t0:t0 + TOK, :].rearrange(
                        "(c p) d -> p c d", p=128)
                    nc.sync.dma_start(xt_f32[:], src)

                    xr = xr_p.tile([128, 2, DIM], F32R, tag="xr")
                    nc.vector.tensor_copy(xr[:], xt_f32[:])

                    # transpose x tile -> xT (128, NCH, TOK) via psum staging
                    xT = xt_p.tile([128, NCH, TOK], F32R, tag="xt")
                    for k0 in range(0, NCH, 2):
                        st = ps_stage.tile([128, 2, 2, 128], F32, tag="stage")
                        for kk in range(2):
                            for blk in range(2):
                                nc.tensor.transpose(
                                    st[:, kk, blk, :],
                                    xt_f32[:, blk, ds((k0 + kk) * 128, 128)],
                                    ident[:])
                        nc.scalar.copy(
                            out=xT[:, k0:k0 + 2, :].rearrange(
                                "p a b -> p (a b)"),
                            in_=st[:].rearrange("p a b c -> p (a b c)"))

                    # pass-1: logitsT = U_b.T @ xT
                    lgt = ps_acc.tile([H, TOK], F32, tag="acc")
                    for ch in range(NCH):
                        nc.tensor.matmul(lgt[:], U[b][:, ch, :], xT[:, ch, :],
                                         start=(ch == 0), stop=(ch == NCH - 1))

                    # exp + per-tile sum
                    at = at_p.tile([H, TOK], F32, tag="at")
                    nc.scalar.activation(out=at[:], in_=lgt[:], func=AF.Exp,
                                         accum_out=Spart[b][:, i:i + 1])

                    # aT -> a (natural) via PE transpose
                    atp = ps_a.tile([128, 2, H], F32, tag="a")
                    for blk in range(2):
                        nc.tensor.transpose(atp[:, blk, :],
                                            at[:, ds(blk * 128, 128)],
                                            ident[0:H, 0:H])
                    a_sb = asb_p.tile([128, 2, H], F32R, tag="asb")
                    nc.vector.tensor_copy(a_sb[:], atp[:])

                    # pass-2: y += a.T @ x
                    for blk in range(2):
                        for hf in range(2):
                            nc.tensor.matmul(
                                yps[:, hf, :], a_sb[:, blk, :],
                                xr[:, blk, ds(hf * 512, 512)],
                                start=(i == 0 and blk == 0),
                                stop=(i == NT - 1 and blk == 1),
                                skip_group_check=True)

                # batch tail: S, ynorm
                s_b = ptmp.tile([H, 1], F32, tag="s")
                nc.vector.reduce_sum(out=s_b[:], in_=Spart[b][:],
                                     axis=mybir.AxisListType.X)
                inv = ptmp.tile([H, 1], F32, tag="inv")
                nc.vector.reciprocal(inv[:], s_b[:])
                for hf in range(2):
                    nc.vector.tensor_scalar_mul(
                        ynorm[b][:, ds(hf * 512, 512)], yps[:, hf, :], inv[:])

            # ---------------- epilogue ----------------
            for b in range(BPC):
                ystage = ps_stage.tile([128, NCH, H], F32, tag="stage")
                for ch in range(NCH):
                    nc.tensor.transpose(ystage[:, ch, :],
                                        ynorm[b][:, ds(ch * 128, 128)],
                                        ident[0:H, 0:H])
                ynr = ptmp.tile([128, NCH, H], F32R, tag="ynr")
                nc.vector.tensor_copy(ynr[:], ystage[:])

                gh = [ps_acc.tile([H, 512], F32, tag="acc", name=f"gh{_}") for _ in range(2)]
                for hf in range(2):
                    for ch in range(NCH):
                        nc.tensor.matmul(gh[hf][:], ynr[:, ch, :],
                                         wvr[:, ch, ds(hf * 512, 512)],
                                         start=(ch == 0), stop=(ch == NCH - 1))
                gm = ptmp.tile([H, DIM], F32R, tag="gm", bufs=1)
                for hf in range(2):
                    nc.vector.tensor_mul(gm[:, ds(hf * 512, 512)], gh[hf][:],
                                         mask[:, ds(hf * 512, 512)])
                zh = [ps_acc.tile([1, 512], F32, tag="acc", name=f"zh{_}") for _ in range(2)]
                for hf in range(2):
                    nc.tensor.matmul(zh[hf][:], ones16[:],
                                     gm[:, ds(hf * 512, 512)],
                                     start=True, stop=True)
                z_sb = ptmp.tile([1, DIM], F32, tag="z", bufs=1)
                for hf in range(2):
                    nc.scalar.copy(out=z_sb[:, ds(hf * 512, 512)], in_=zh[hf][:])

                ztp = ps_a.tile([128, NCH], F32, tag="a")
                for ch in range(NCH):
                    nc.tensor.transpose(ztp[:, ch:ch + 1],
                                        z_sb[0:1, ds(ch * 128, 128)],
                                        ident[0:1, 0:1])
                zt = ptmp.tile([128, NCH], F32R, tag="zt")
                nc.vector.tensor_copy(zt[:], ztp[:])

                oh = [ps_acc.tile([1, 512], F32, tag="acc", name=f"oh{_}") for _ in range(2)]
                for hf in range(2):
                    for ch in range(NCH):
                        nc.tensor.matmul(oh[hf][:], zt[:, ch:ch + 1],
                                         wpr[:, ch, ds(hf * 512, 512)],
                                         start=(ch == 0), stop=(ch == NCH - 1))
                ob = ptmp.tile([1, DIM], F32, tag="ob")
                for hf in range(2):
                    nc.vector.tensor_add(ob[:, ds(hf * 512, 512)], oh[hf][:],
                                         bp_sb[:, ds(hf * 512, 512)])
                nc.sync.dma_start(T["out"][b:b + 1, :], ob[:])


def _build():
    if "nc" in _CACHE:
        return _CACHE["nc"]
    nc = bacc.Bacc("TRN2", target_bir_lowering=False, debug=False,
                   num_devices=NCORES)
    T = {}
    T["x"] = nc.dram_tensor("x", [BPC, N, DIM], F32, kind="ExternalInput").ap()
    T["x0t"] = nc.dram_tensor("x0t", [128, NCH, BPC], F32,
                              kind="ExternalInput").ap()
    for w in ("wq", "wkt", "wv", "wp"):
        T[w] = nc.dram_tensor(w, [DIM, DIM], F32, kind="ExternalInput").ap()
    T["bp"] = nc.dram_tensor("bp", [1, DIM], F32, kind="ExternalInput").ap()
    for t_ in ("cosq", "sinq", "cose", "sine"):
        T[t_] = nc.dram_tensor(t_, [BPC, DIM], F32, kind="ExternalInput").ap()
    T["mask"] = nc.dram_tensor("mask", [H, DIM], F32, kind="ExternalInput").ap()
    T["out"] = nc.dram_tensor("out", [BPC, DIM], F32, kind="ExternalOutput").ap()

    with tile.TileContext(nc) as tc:
        _emit(tc, T)
    nc.compile()
    _CACHE["nc"] = nc
    return nc


# ------------------------------------------------------------------ host side
def _in_maps(x, Wq, Wk, Wv, Wp, bp):
    cosq, sinq, cose, sine, mask = _host_tables()
    wkt = np.ascontiguousarray(Wk.T)
    bp1 = np.ascontiguousarray(bp.reshape(1, DIM))
    maps = []
    for c in range(NCORES):
        xs = np.ascontiguousarray(x[BPC * c:BPC * (c + 1)])
        x0 = xs[:, 0, :]                                     # (BPC, DIM)
        x0t = np.ascontiguousarray(
            x0.T.reshape(NCH, 128, BPC).transpose(1, 0, 2))  # (128, NCH, BPC)
        maps.append({"x": xs, "x0t": x0t, "wq": Wq, "wkt": wkt, "wv": Wv,
                     "wp": Wp, "bp": bp1, "cosq": cosq, "sinq": sinq,
                     "cose": cose, "sine": sine, "mask": mask})
    return maps


def run(x, Wq, Wk, Wv, Wp, bp, **kwargs):
    nc = _build()
    maps = _in_maps(x, Wq, Wk, Wv, Wp, bp)
    res = run_bass_kernel_spmd(nc, maps, core_ids=list(range(NCORES)), **kwargs)
    out = np.stack([r["out"] for r in res.results])          # (8, BPC, DIM)
    return out.reshape(B, 1, DIM), res


def kernel(x, Wq, Wk, Wv, Wp, bp):
    x = np.ascontiguousarray(np.asarray(x), dtype=np.float32)
    Wq = np.ascontiguousarray(np.asarray(Wq), dtype=np.float32)
    Wk = np.ascontiguousarray(np.asarray(Wk), dtype=np.float32)
    Wv = np.ascontiguousarray(np.asarray(Wv), dtype=np.float32)
    Wp = np.ascontiguousarray(np.asarray(Wp), dtype=np.float32)
    bp = np.ascontiguousarray(np.asarray(bp), dtype=np.float32)
    out, _ = run(x, Wq, Wk, Wv, Wp, bp)
    return out


# revision 4
# speedup vs baseline: 1.1601x; 1.1601x over previous
"""Trainium2 Bass kernel for nn_CrossAttention_70866960384676.

Reference semantics: cross-attention where only token 0 of each batch is the
query; K/V projections span the full sequence; rotary uses head-index
positions (constant over sequence).

Algebraic reduction (validated vs reference at ~1e-6 rel in fp32):
  q_rot = rotary(x0 @ Wq);  e = rotary_adjoint(q_rot) * DH^-0.5
  U[:, h] = Wk[:, h*DH:(h+1)*DH] @ e[h]          (per batch; 1024x16)
  logits = x @ U                                  (N x H)
  a = exp(logits);  S = sum_n a
  ynorm = (a.T @ x) / S                           (H x 1024)
  z[h*DH:(h+1)*DH] = ynorm[h] @ Wv[:, h*DH:(h+1)*DH]
  out = z @ Wp + bp

This turns a 275-GFLOP dense problem into a DMA-bound streaming problem
(~50 MB/core).  Sharding: pure data-parallel, 2 batches per core, 8 cores.

On-chip structure per core (hot matmuls in float32r; transposes in fp32):
  pass-1 needs x with dim on partitions -> PE transposes of each x tile
  pass-2 consumes x in natural layout (f32r-rounded)
"""
import numpy as np
from contextlib import ExitStack

import concourse.bass as bass
import concourse.tile as tile
from concourse import bacc, mybir
from concourse.bass import ds
from concourse.bass_utils import run_bass_kernel_spmd
from concourse.masks import make_identity

dt = mybir.dt
F32 = dt.float32
F32R = dt.float32r
F16 = dt.float16
AF = mybir.ActivationFunctionType

B, N, DIM, H, DH = 16, 4096, 1024, 16, 64
NCORES = 8
BPC = B // NCORES          # batches per core
NCH = DIM // 128           # 8 dim chunks
TOK = 512                  # tokens per main-loop tile
NT = N // TOK              # 16 tiles per batch
THETA = 10000.0
SCALE = DH ** -0.5

_CACHE = {}


# ---------------------------------------------------------------- host tables
def _host_tables():
    inv = 1.0 / (THETA ** (np.arange(0, DH, 2, dtype=np.float64) / DH))
    t = np.arange(H, dtype=np.float64)
    fr = t[:, None] * inv[None, :]
    emb = np.concatenate([fr, fr], -1)                      # (H, DH)
    cos = np.cos(emb).reshape(1, DIM)
    sin = np.sin(emb).reshape(1, DIM)
    cosq = np.broadcast_to(cos, (BPC, DIM)).astype(np.float32).copy()
    sinq = np.broadcast_to(sin, (BPC, DIM)).astype(np.float32).copy()
    cose = (cosq * SCALE).astype(np.float32)
    sine = (sinq * SCALE).astype(np.float32)
    mask = np.zeros((H, DIM), np.float32)                   # head-block mask
    for h in range(H):
        mask[h, h * DH:(h + 1) * DH] = 1.0
    return cosq, sinq, cose, sine, mask


# ------------------------------------------------------------------ bass emit
def _emit(tc, T):
    nc = tc.nc
    with ExitStack() as ctx:
        persist = ctx.enter_context(tc.tile_pool(name="persist", bufs=1))
        ptmp = ctx.enter_context(tc.tile_pool(name="ptmp", bufs=2))
        wtmp = ctx.enter_context(tc.tile_pool(name="wtmp", bufs=2))
        xr_p = ctx.enter_context(tc.tile_pool(name="xr", bufs=3))
        xt_p = ctx.enter_context(tc.tile_pool(name="xt", bufs=2))
        at_p = ctx.enter_context(tc.tile_pool(name="at", bufs=2))
        asb_p = ctx.enter_context(tc.tile_pool(name="asb", bufs=2))
        ps_stage = ctx.enter_context(
            tc.tile_pool(name="ps_stage", bufs=3, space="PSUM"))
        ps_acc = ctx.enter_context(
            tc.tile_pool(name="ps_acc", bufs=2, space="PSUM"))
        ps_a = ctx.enter_context(tc.tile_pool(name="ps_a", bufs=1, space="PSUM"))
        ps_y = ctx.enter_context(tc.tile_pool(name="ps_y", bufs=1, space="PSUM"))

        # ---------------- constants ----------------
        ident = persist.tile([128, 128], F32)
        make_identity(nc, ident)
        identh = persist.tile([128, 128], F16)
        nc.vector.tensor_copy(identh[:], ident[:])
        cosq = persist.tile([BPC, DIM], F32)
        sinq = persist.tile([BPC, DIM], F32)
        cose = persist.tile([BPC, DIM], F32)
        sine = persist.tile([BPC, DIM], F32)
        mask = persist.tile([H, DIM], F32)
        bp_sb = persist.tile([1, DIM], F32)
        x0t_sb = persist.tile([128, NCH, BPC], F32)
        for name, t_ in [("cosq", cosq), ("sinq", sinq), ("cose", cose),
                         ("sine", sine), ("mask", mask), ("bp", bp_sb),
                         ("x0t", x0t_sb)]:
            nc.sync.dma_start(t_[:], T[name])

        ones16f = persist.tile([H, 1], F32)
        nc.vector.memset(ones16f[:], 1.0)
        ones16 = persist.tile([H, 1], F32R)
        nc.vector.tensor_copy(ones16[:], ones16f[:])

        x0r = persist.tile([128, NCH, BPC], F32R)
        nc.vector.tensor_copy(x0r[:], x0t_sb[:])

        # ---------------- prologue: weights Wq / WkT ----------------
        def load_weight_rounded(dram_ap, pool, tag):
            w = pool.tile([128, NCH, DIM], F32R, tag=tag)
            for ch in range(NCH):
                tmp = wtmp.tile([128, DIM], F32, tag="wtmp")
                nc.sync.dma_start(tmp[:], dram_ap[ch * 128:(ch + 1) * 128, :])
                nc.vector.tensor_copy(w[:, ch, :], tmp[:])
            return w

        with tc.tile_pool(name="w1", bufs=1) as w1, \
                tc.tile_pool(name="rot", bufs=3) as rot:
            wqr = load_weight_rounded(T["wq"], w1, "wbig")

            # q = x0 @ Wq  -> psum (BPC, 1024) in two halves
            qh = [ps_acc.tile([BPC, 512], F32, tag="acc", name=f"qh{_}") for _ in range(2)]
            for hf in range(2):
                for ch in range(NCH):
                    nc.tensor.matmul(qh[hf][:], x0r[:, ch, :],
                                     wqr[:, ch, ds(hf * 512, 512)],
                                     start=(ch == 0), stop=(ch == NCH - 1))
            q_sb = rot.tile([BPC, DIM], F32, tag="rot")
            for hf in range(2):
                nc.scalar.copy(out=q_sb[:, ds(hf * 512, 512)], in_=qh[hf][:])

            # rotary on q (view (BPC, H, DH))
            def halves(t_):
                v = t_[:].rearrange("p (h d) -> p h d", h=H)
                return v[:, :, 0:DH // 2], v[:, :, DH // 2:DH]

            rh = rot.tile([BPC, DIM], F32, tag="rot")
            rh_lo, rh_hi = halves(rh)
            q_lo, q_hi = halves(q_sb)
            nc.vector.tensor_scalar_mul(rh_lo, q_hi, -1.0)
            nc.vector.tensor_copy(rh_hi, q_lo)
            t1 = rot.tile([BPC, DIM], F32, tag="rot")
            nc.vector.tensor_mul(t1[:], q_sb[:], cosq[:])
            t2 = rot.tile([BPC, DIM], F32, tag="rot")
            nc.vector.tensor_mul(t2[:], rh[:], sinq[:])
            qrot = rot.tile([BPC, DIM], F32, tag="rot")
            nc.vector.tensor_add(qrot[:], t1[:], t2[:])
            # e = qrot*cose + rotadj(qrot*sine)   (cose/sine carry the 1/8)
            y2 = rot.tile([BPC, DIM], F32, tag="rot")
            nc.vector.tensor_mul(y2[:], qrot[:], sine[:])
            radj = rot.tile([BPC, DIM], F32, tag="rot")
            ra_lo, ra_hi = halves(radj)
            y2_lo, y2_hi = halves(y2)
            nc.vector.tensor_copy(ra_lo, y2_hi)
            nc.vector.tensor_scalar_mul(ra_hi, y2_lo, -1.0)
            t3 = rot.tile([BPC, DIM], F32, tag="rot")
            nc.vector.tensor_mul(t3[:], qrot[:], cose[:])
            e_sb = rot.tile([BPC, DIM], F32, tag="rot")
            nc.vector.tensor_add(e_sb[:], t3[:], radj[:])

            # e -> eT (128, NCH, BPC)
            etp = ps_stage.tile([128, NCH, BPC], F32, tag="stage")
            for ch in range(NCH):
                nc.tensor.transpose(etp[:, ch, :], e_sb[:, ds(ch * 128, 128)],
                                    ident[0:BPC, 0:BPC])
            eT = persist.tile([128, NCH, BPC], F32)
            nc.vector.tensor_copy(eT[:], etp[:])

            # E_b block-diagonal (128, NCH, H), then U_b = (E_b.T @ WkT).T
            wktr = load_weight_rounded(T["wkt"], w1, "wbig")
            U = []
            for b in range(BPC):
                ef = persist.tile([128, NCH, H], F32, tag=f"ef{b}")
                nc.vector.memset(ef[:], 0.0)
                eflat = ef[:].rearrange("p a b -> p (a b)")
                nc.vector.tensor_copy(eflat[0:64, 0:127:18], eT[0:64, :, b])
                nc.vector.tensor_copy(eflat[64:128, 1:128:18], eT[64:128, :, b])
                er = persist.tile([128, NCH, H], F32R, tag=f"er{b}")
                nc.vector.tensor_copy(er[:], ef[:])

                uth = [ps_acc.tile([H, 512], F32, tag="acc", name=f"uth{_}") for _ in range(2)]
                for hf in range(2):
                    for ch in range(NCH):
                        nc.tensor.matmul(uth[hf][:], er[:, ch, :],
                                         wktr[:, ch, ds(hf * 512, 512)],
                                         start=(ch == 0), stop=(ch == NCH - 1))
                utr = ptmp.tile([H, DIM], F32, tag="utr", bufs=1)
                for hf in range(2):
                    nc.scalar.copy(out=utr[:, ds(hf * 512, 512)], in_=uth[hf][:])
                ustage = ps_stage.tile([128, NCH, H], F32, tag="stage")
                for ch in range(NCH):
                    nc.tensor.transpose(ustage[:, ch, :],
                                        utr[:, ds(ch * 128, 128)],
                                        ident[0:H, 0:H])
                u_b = persist.tile([128, NCH, H], F16, tag=f"u{b}")
                nc.vector.tensor_copy(u_b[:], ustage[:])
                U.append(u_b)

        # ---------------- main loop ----------------
        Spart = [persist.tile([H, NT], F32, tag=f"sp{b}", name=f"sp{b}") for b in range(BPC)]
        ynorm = [persist.tile([H, DIM], F32, tag=f"yn{b}", name=f"yn{b}") for b in range(BPC)]

        with tc.tile_pool(name="w2", bufs=1) as w2:
            wvr = load_weight_rounded(T["wv"], w2, "wvr")
            wpr = load_weight_rounded(T["wp"], w2, "wpr")

            NB = TOK // 128            # 128-token blocks per tile
            for b in range(BPC):
                yps = ps_y.tile([H, 2, 512], F32, tag="y")
                for i in range(NT):
                    t0 = i * TOK
                    # casting DMA (SWDGE): DRAM fp32 -> SBUF fp16
                    xr = xr_p.tile([128, NB, DIM], F16, tag="xr")
                    src = T["x"][b, t0:t0 + TOK, :].rearrange(
                        "(c p) d -> p c d", p=128)
                    nc.gpsimd.dma_start(xr[:], src)

                    # transpose x tile -> xT (128, NCH, TOK) via psum staging
                    xT = xt_p.tile([128, NCH, TOK], F16, tag="xt")
                    for k0 in range(0, NCH, 2):
                        st = ps_stage.tile([128, 2, NB, 128], F16, tag="stage")
                        for kk in range(2):
                            for blk in range(NB):
                                nc.tensor.transpose(
                                    st[:, kk, blk, :],
                                    xr[:, blk, ds((k0 + kk) * 128, 128)],
                                    identh[:])
                        eng = nc.scalar if (k0 // 2) % 2 == 0 else nc.vector
                        if eng is nc.scalar:
                            nc.scalar.copy(
                                out=xT[:, k0:k0 + 2, :].rearrange(
                                    "p a b -> p (a b)"),
                                in_=st[:].rearrange("p a b c -> p (a b c)"))
                        else:
                            nc.vector.tensor_copy(
                                xT[:, k0:k0 + 2, :].rearrange(
                                    "p a b -> p (a b)"),
                                st[:].rearrange("p a b c -> p (a b c)"))

                    # pass-1: logitsT = U_b.T @ xT
                    lgt = ps_acc.tile([H, TOK], F32, tag="acc")
                    for ch in range(NCH):
                        nc.tensor.matmul(lgt[:], U[b][:, ch, :], xT[:, ch, :],
                                         start=(ch == 0), stop=(ch == NCH - 1))

                    # exp + per-tile sum
                    at = at_p.tile([H, TOK], F16, tag="at")
                    nc.scalar.activation(out=at[:], in_=lgt[:], func=AF.Exp,
                                         accum_out=Spart[b][:, i:i + 1])

                    # aT -> a (natural) via PE transpose
                    atp = ps_a.tile([128, NB, H], F16, tag="a")
                    for blk in range(NB):
                        nc.tensor.transpose(atp[:, blk, :],
                                            at[:, ds(blk * 128, 128)],
                                            identh[0:H, 0:H])
                    a_sb = asb_p.tile([128, NB, H], F16, tag="asb")
                    nc.vector.tensor_copy(a_sb[:], atp[:])

                    # pass-2: y += a.T @ x
                    for blk in range(NB):
                        for hf in range(2):
                            nc.tensor.matmul(
                                yps[:, hf, :], a_sb[:, blk, :],
                                xr[:, blk, ds(hf * 512, 512)],
                                start=(i == 0 and blk == 0),
                                stop=(i == NT - 1 and blk == NB - 1),
                                skip_group_check=True)

                # batch tail: S, ynorm
                s_b = ptmp.tile([H, 1], F32, tag="s")
                nc.vector.reduce_sum(out=s_b[:], in_=Spart[b][:],
                                     axis=mybir.AxisListType.X)
                inv = ptmp.tile([H, 1], F32, tag="inv")
                nc.vector.reciprocal(inv[:], s_b[:])
                for hf in range(2):
                    nc.vector.tensor_scalar_mul(
                        ynorm[b][:, ds(hf * 512, 512)], yps[:, hf, :], inv[:])

            # ---------------- epilogue ----------------
            for b in range(BPC):
                ystage = ps_stage.tile([128, NCH, H], F32, tag="stage")
                for ch in range(NCH):
                    nc.tensor.transpose(ystage[:, ch, :],
                                        ynorm[b][:, ds(ch * 128, 128)],
                                        ident[0:H, 0:H])
                ynr = ptmp.tile([128, NCH, H], F32R, tag="ynr")
                nc.vector.tensor_copy(ynr[:], ystage[:])

                gh = [ps_acc.tile([H, 512], F32, tag="acc", name=f"gh{_}") for _ in range(2)]
                for hf in range(2):
                    for ch in range(NCH):
                        nc.tensor.matmul(gh[hf][:], ynr[:, ch, :],
                                         wvr[:, ch, ds(hf * 512, 512)],
                                         start=(ch == 0), stop=(ch == NCH - 1))
                gm = ptmp.tile([H, DIM], F32R, tag="gm", bufs=1)
                for hf in range(2):
                    nc.vector.tensor_mul(gm[:, ds(hf * 512, 512)], gh[hf][:],
                                         mask[:, ds(hf * 512, 512)])
                zh = [ps_acc.tile([1, 512], F32, tag="acc", name=f"zh{_}") for _ in range(2)]
                for hf in range(2):
                    nc.tensor.matmul(zh[hf][:], ones16[:],
                                     gm[:, ds(hf * 512, 512)],
                                     start=True, stop=True)
                z_sb = ptmp.tile([1, DIM], F32, tag="z", bufs=1)
                for hf in range(2):
                    nc.scalar.copy(out=z_sb[:, ds(hf * 512, 512)], in_=zh[hf][:])

                ztp = ps_a.tile([128, NCH], F32, tag="a")
                for ch in range(NCH):
                    nc.tensor.transpose(ztp[:, ch:ch + 1],
                                        z_sb[0:1, ds(ch * 128, 128)],
                                        ident[0:1, 0:1])
                zt = ptmp.tile([128, NCH], F32R, tag="zt")
                nc.vector.tensor_copy(zt[:], ztp[:])

                oh = [ps_acc.tile([1, 512], F32, tag="acc", name=f"oh{_}") for _ in range(2)]
                for hf in range(2):
                    for ch in range(NCH):
                        nc.tensor.matmul(oh[hf][:], zt[:, ch:ch + 1],
                                         wpr[:, ch, ds(hf * 512, 512)],
                                         start=(ch == 0), stop=(ch == NCH - 1))
                ob = ptmp.tile([1, DIM], F32, tag="ob")
                for hf in range(2):
                    nc.vector.tensor_add(ob[:, ds(hf * 512, 512)], oh[hf][:],
                                         bp_sb[:, ds(hf * 512, 512)])
                nc.sync.dma_start(T["out"][b:b + 1, :], ob[:])


def _build():
    if "nc" in _CACHE:
        return _CACHE["nc"]
    nc = bacc.Bacc("TRN2", target_bir_lowering=False, debug=False,
                   num_devices=NCORES)
    T = {}
    T["x"] = nc.dram_tensor("x", [BPC, N, DIM], F32, kind="ExternalInput").ap()
    T["x0t"] = nc.dram_tensor("x0t", [128, NCH, BPC], F32,
                              kind="ExternalInput").ap()
    for w in ("wq", "wkt", "wv", "wp"):
        T[w] = nc.dram_tensor(w, [DIM, DIM], F32, kind="ExternalInput").ap()
    T["bp"] = nc.dram_tensor("bp", [1, DIM], F32, kind="ExternalInput").ap()
    for t_ in ("cosq", "sinq", "cose", "sine"):
        T[t_] = nc.dram_tensor(t_, [BPC, DIM], F32, kind="ExternalInput").ap()
    T["mask"] = nc.dram_tensor("mask", [H, DIM], F32, kind="ExternalInput").ap()
    T["out"] = nc.dram_tensor("out", [BPC, DIM], F32, kind="ExternalOutput").ap()

    with tile.TileContext(nc) as tc:
        _emit(tc, T)
    nc.compile()
    _CACHE["nc"] = nc
    return nc


# ------------------------------------------------------------------ host side
def _in_maps(x, Wq, Wk, Wv, Wp, bp):
    cosq, sinq, cose, sine, mask = _host_tables()
    wkt = np.ascontiguousarray(Wk.T)
    bp1 = np.ascontiguousarray(bp.reshape(1, DIM))
    maps = []
    for c in range(NCORES):
        xs = np.ascontiguousarray(x[BPC * c:BPC * (c + 1)])
        x0 = xs[:, 0, :]                                     # (BPC, DIM)
        x0t = np.ascontiguousarray(
            x0.T.reshape(NCH, 128, BPC).transpose(1, 0, 2))  # (128, NCH, BPC)
        maps.append({"x": xs, "x0t": x0t, "wq": Wq, "wkt": wkt, "wv": Wv,
                     "wp": Wp, "bp": bp1, "cosq": cosq, "sinq": sinq,
                     "cose": cose, "sine": sine, "mask": mask})
    return maps


def run(x, Wq, Wk, Wv, Wp, bp, **kwargs):
    nc = _build()
    maps = _in_maps(x, Wq, Wk, Wv, Wp, bp)
    res = run_bass_kernel_spmd(nc, maps, core_ids=list(range(NCORES)), **kwargs)
    out = np.stack([r["out"] for r in res.results])          # (8, BPC, DIM)
    return out.reshape(B, 1, DIM), res


def kernel(x, Wq, Wk, Wv, Wp, bp):
    x = np.ascontiguousarray(np.asarray(x), dtype=np.float32)
    Wq = np.ascontiguousarray(np.asarray(Wq), dtype=np.float32)
    Wk = np.ascontiguousarray(np.asarray(Wk), dtype=np.float32)
    Wv = np.ascontiguousarray(np.asarray(Wv), dtype=np.float32)
    Wp = np.ascontiguousarray(np.asarray(Wp), dtype=np.float32)
    bp = np.ascontiguousarray(np.asarray(bp), dtype=np.float32)
    out, _ = run(x, Wq, Wk, Wv, Wp, bp)
    return out


# revision 6
# speedup vs baseline: 1.1629x; 1.0025x over previous
"""Trainium2 Bass kernel for nn_CrossAttention_70866960384676.

Reference semantics: cross-attention where only token 0 of each batch is the
query; K/V projections span the full sequence; rotary uses head-index
positions (constant over sequence).

Algebraic reduction (validated vs reference at ~1e-6 rel in fp32):
  q_rot = rotary(x0 @ Wq);  e = rotary_adjoint(q_rot) * DH^-0.5
  U[:, h] = Wk[:, h*DH:(h+1)*DH] @ e[h]          (per batch; 1024x16)
  logits = x @ U                                  (N x H)
  a = exp(logits);  S = sum_n a
  ynorm = (a.T @ x) / S                           (H x 1024)
  z[h*DH:(h+1)*DH] = ynorm[h] @ Wv[:, h*DH:(h+1)*DH]
  out = z @ Wp + bp

This turns a 275-GFLOP dense problem into a DMA-bound streaming problem
(~50 MB/core).  Sharding: pure data-parallel, 2 batches per core, 8 cores.

On-chip structure per core (hot matmuls in float32r; transposes in fp32):
  pass-1 needs x with dim on partitions -> PE transposes of each x tile
  pass-2 consumes x in natural layout (f32r-rounded)
"""
import numpy as np
from contextlib import ExitStack

import concourse.bass as bass
import concourse.tile as tile
from concourse import bacc, mybir
from concourse.bass import ds
from concourse.bass_utils import run_bass_kernel_spmd
from concourse.masks import make_identity

dt = mybir.dt
F32 = dt.float32
F32R = dt.float32r
F16 = dt.float16
AF = mybir.ActivationFunctionType

B, N, DIM, H, DH = 16, 4096, 1024, 16, 64
NCORES = 8
BPC = B // NCORES          # batches per core
NCH = DIM // 128           # 8 dim chunks
TOK = 512                  # tokens per main-loop tile
NT = N // TOK              # 16 tiles per batch
THETA = 10000.0
SCALE = DH ** -0.5

_CACHE = {}


# ---------------------------------------------------------------- host tables
def _host_tables():
    inv = 1.0 / (THETA ** (np.arange(0, DH, 2, dtype=np.float64) / DH))
    t = np.arange(H, dtype=np.float64)
    fr = t[:, None] * inv[None, :]
    emb = np.concatenate([fr, fr], -1)                      # (H, DH)
    c = np.cos(emb).reshape(DIM)
    sn = np.sin(emb).reshape(DIM)
    # combined rotary + adjoint + scale linear map, block-diag per head:
    # e = L @ q  where L = R2 @ R1 (see reference rotary semantics)
    L = np.zeros((DIM, DIM))
    hw = DH // 2
    for h in range(H):
        sl = slice(h * DH, (h + 1) * DH)
        cb = np.diag(c[sl])
        sb = np.diag(sn[sl])
        Rh = np.zeros((DH, DH))
        J = np.zeros((DH, DH))
        for i in range(hw):
            Rh[i, i + hw] = -1
            Rh[i + hw, i] = 1
            J[i, i + hw] = 1
            J[i + hw, i] = -1
        L[sl, sl] = ((cb + J @ sb) * SCALE) @ (cb + sb @ Rh)
    # lhsT chunks for e = L @ qT:  lt[p, ch, m] = L[ch*128+m, ch*128+p]
    lt = np.zeros((128, NCH, 128), np.float32)
    for ch in range(NCH):
        blk = L[ch * 128:(ch + 1) * 128, ch * 128:(ch + 1) * 128]
        lt[:, ch, :] = blk.T.astype(np.float32)
    mask = np.zeros((H, DIM), np.float32)                   # head-block mask
    for h in range(H):
        mask[h, h * DH:(h + 1) * DH] = 1.0
    return lt, mask


# ------------------------------------------------------------------ bass emit
def _emit(tc, T):
    nc = tc.nc
    with ExitStack() as ctx:
        persist = ctx.enter_context(tc.tile_pool(name="persist", bufs=1))
        ptmp = ctx.enter_context(tc.tile_pool(name="ptmp", bufs=2))
        wtmp = ctx.enter_context(tc.tile_pool(name="wtmp", bufs=2))
        xr_p = ctx.enter_context(tc.tile_pool(name="xr", bufs=3))
        xt_p = ctx.enter_context(tc.tile_pool(name="xt", bufs=2))
        at_p = ctx.enter_context(tc.tile_pool(name="at", bufs=2))
        asb_p = ctx.enter_context(tc.tile_pool(name="asb", bufs=2))
        ps_stage = ctx.enter_context(
            tc.tile_pool(name="ps_stage", bufs=3, space="PSUM"))
        ps_acc = ctx.enter_context(
            tc.tile_pool(name="ps_acc", bufs=2, space="PSUM"))
        ps_a = ctx.enter_context(tc.tile_pool(name="ps_a", bufs=1, space="PSUM"))
        ps_y = ctx.enter_context(tc.tile_pool(name="ps_y", bufs=1, space="PSUM"))

        # ---------------- constants ----------------
        ident = persist.tile([128, 128], F32)
        make_identity(nc, ident)
        identh = persist.tile([128, 128], F16)
        nc.vector.tensor_copy(identh[:], ident[:])
        mask = persist.tile([H, DIM], F32)
        bp_sb = persist.tile([1, DIM], F32)
        x0t_sb = persist.tile([128, NCH, BPC], F32)
        lt_sb = persist.tile([128, NCH, 128], F32)
        for name, t_ in [("mask", mask), ("bp", bp_sb), ("x0t", x0t_sb),
                         ("lt", lt_sb)]:
            nc.sync.dma_start(t_[:], T[name])

        ones16f = persist.tile([H, 1], F32)
        nc.vector.memset(ones16f[:], 1.0)
        ones16 = persist.tile([H, 1], F32R)
        nc.vector.tensor_copy(ones16[:], ones16f[:])

        x0r = persist.tile([128, NCH, BPC], F32R)
        nc.vector.tensor_copy(x0r[:], x0t_sb[:])

        # ---------------- prologue: weights Wq / WkT ----------------
        def load_weight_rounded(dram_ap, pool, tag, eng=None):
            eng = eng or nc.sync
            w = pool.tile([128, NCH, DIM], F32R, tag=tag)
            for ch in range(NCH):
                tmp = wtmp.tile([128, DIM], F32, tag="wtmp")
                eng.dma_start(tmp[:], dram_ap[ch * 128:(ch + 1) * 128, :])
                nc.vector.tensor_copy(w[:, ch, :], tmp[:])
            return w

        with tc.tile_pool(name="w1", bufs=1) as w1:
            wktr = load_weight_rounded(T["wkt"], w1, "wkt", eng=nc.scalar)
            wqr = load_weight_rounded(T["wq"], w1, "wq", eng=nc.sync)

            # q = x0 @ Wq  -> psum (BPC, 1024) in two halves
            qh = [ps_acc.tile([BPC, 512], F32, tag="acc", name=f"qh{_}") for _ in range(2)]
            for ch in range(NCH):
                for hf in range(2):
                    nc.tensor.matmul(qh[hf][:], x0r[:, ch, :],
                                     wqr[:, ch, ds(hf * 512, 512)],
                                     start=(ch == 0), stop=(ch == NCH - 1))
            q_sb = ptmp.tile([BPC, DIM], F32, tag="qsb", bufs=1)
            for hf in range(2):
                nc.scalar.copy(out=q_sb[:, ds(hf * 512, 512)], in_=qh[hf][:])

            # qT via PE transposes, then e = L @ qT (fp32, block-diag L)
            qtp = ps_stage.tile([128, NCH, BPC], F32, tag="stage")
            for ch in range(NCH):
                nc.tensor.transpose(qtp[:, ch, :], q_sb[:, ds(ch * 128, 128)],
                                    ident[0:BPC, 0:BPC])
            qT = ptmp.tile([128, NCH, BPC], F32, tag="qt", bufs=1)
            nc.vector.tensor_copy(qT[:], qtp[:])
            eTp = ps_a.tile([128, NCH, BPC], F32, tag="a")
            for ch in range(NCH):
                nc.tensor.matmul(eTp[:, ch, :], lt_sb[:, ch, :], qT[:, ch, :],
                                 start=True, stop=True)
            eT = persist.tile([128, NCH, BPC], F32)
            nc.vector.tensor_copy(eT[:], eTp[:])

            # E_b block-diagonal (128, NCH, H), then U_b = (E_b.T @ WkT).T
            U = []
            for b in range(BPC):
                ef = persist.tile([128, NCH, H], F32, tag=f"ef{b}")
                nc.vector.memset(ef[:], 0.0)
                eflat = ef[:].rearrange("p a b -> p (a b)")
                nc.vector.tensor_copy(eflat[0:64, 0:127:18], eT[0:64, :, b])
                nc.vector.tensor_copy(eflat[64:128, 1:128:18], eT[64:128, :, b])
                er = persist.tile([128, NCH, H], F32R, tag=f"er{b}")
                nc.vector.tensor_copy(er[:], ef[:])

                uth = [ps_acc.tile([H, 512], F32, tag="acc", name=f"uth{_}") for _ in range(2)]
                for ch in range(NCH):
                    for hf in range(2):
                        nc.tensor.matmul(uth[hf][:], er[:, ch, :],
                                         wktr[:, ch, ds(hf * 512, 512)],
                                         start=(ch == 0), stop=(ch == NCH - 1))
                utr = ptmp.tile([H, DIM], F32, tag="utr", bufs=1)
                for hf in range(2):
                    nc.scalar.copy(out=utr[:, ds(hf * 512, 512)], in_=uth[hf][:])
                ustage = ps_stage.tile([128, NCH, H], F32, tag="stage")
                for ch in range(NCH):
                    nc.tensor.transpose(ustage[:, ch, :],
                                        utr[:, ds(ch * 128, 128)],
                                        ident[0:H, 0:H])
                u_b = persist.tile([128, NCH, H], F16, tag=f"u{b}")
                nc.vector.tensor_copy(u_b[:], ustage[:])
                U.append(u_b)

        # ---------------- main loop ----------------
        Spart = [persist.tile([H, NT], F32, tag=f"sp{b}", name=f"sp{b}") for b in range(BPC)]
        ynorm = [persist.tile([H, DIM], F32, tag=f"yn{b}", name=f"yn{b}") for b in range(BPC)]

        with tc.tile_pool(name="w2", bufs=1) as w2:
            wvr = load_weight_rounded(T["wv"], w2, "wvr", eng=nc.sync)
            wpr = load_weight_rounded(T["wp"], w2, "wpr", eng=nc.scalar)

            NB = TOK // 128            # 128-token blocks per tile
            for b in range(BPC):
                yps = ps_y.tile([H, 2, 512], F32, tag="y")
                for i in range(NT):
                    t0 = i * TOK
                    # casting DMA (SWDGE): DRAM fp32 -> SBUF fp16
                    xr = xr_p.tile([128, NB, DIM], F16, tag="xr")
                    src = T["x"][b, t0:t0 + TOK, :].rearrange(
                        "(c p) d -> p c d", p=128)
                    nc.gpsimd.dma_start(xr[:], src)

                    # transpose x tile -> xT (128, NCH, TOK) via psum staging
                    xT = xt_p.tile([128, NCH, TOK], F16, tag="xt")
                    for k0 in range(0, NCH, 2):
                        st = ps_stage.tile([128, 2, NB, 128], F16, tag="stage")
                        for kk in range(2):
                            for blk in range(NB):
                                nc.tensor.transpose(
                                    st[:, kk, blk, :],
                                    xr[:, blk, ds((k0 + kk) * 128, 128)],
                                    identh[:])
                        eng = nc.scalar if (k0 // 2) % 2 == 0 else nc.vector
                        if eng is nc.scalar:
                            nc.scalar.copy(
                                out=xT[:, k0:k0 + 2, :].rearrange(
                                    "p a b -> p (a b)"),
                                in_=st[:].rearrange("p a b c -> p (a b c)"))
                        else:
                            nc.vector.tensor_copy(
                                xT[:, k0:k0 + 2, :].rearrange(
                                    "p a b -> p (a b)"),
                                st[:].rearrange("p a b c -> p (a b c)"))

                    # pass-1: logitsT = U_b.T @ xT
                    lgt = ps_acc.tile([H, TOK], F32, tag="acc")
                    for ch in range(NCH):
                        nc.tensor.matmul(lgt[:], U[b][:, ch, :], xT[:, ch, :],
                                         start=(ch == 0), stop=(ch == NCH - 1))

                    # exp + per-tile sum
                    at = at_p.tile([H, TOK], F16, tag="at")
                    nc.scalar.activation(out=at[:], in_=lgt[:], func=AF.Exp,
                                         accum_out=Spart[b][:, i:i + 1])

                    # aT -> a (natural) via PE transpose
                    atp = ps_a.tile([128, NB, H], F16, tag="a")
                    for blk in range(NB):
                        nc.tensor.transpose(atp[:, blk, :],
                                            at[:, ds(blk * 128, 128)],
                                            identh[0:H, 0:H])
                    a_sb = asb_p.tile([128, NB, H], F16, tag="asb")
                    nc.vector.tensor_copy(a_sb[:], atp[:])

                    # pass-2: y += a.T @ x
                    for blk in range(NB):
                        for hf in range(2):
                            nc.tensor.matmul(
                                yps[:, hf, :], a_sb[:, blk, :],
                                xr[:, blk, ds(hf * 512, 512)],
                                start=(i == 0 and blk == 0),
                                stop=(i == NT - 1 and blk == NB - 1),
                                skip_group_check=True)

                # batch tail: S, ynorm
                s_b = ptmp.tile([H, 1], F32, tag="s")
                nc.vector.reduce_sum(out=s_b[:], in_=Spart[b][:],
                                     axis=mybir.AxisListType.X)
                inv = ptmp.tile([H, 1], F32, tag="inv")
                nc.vector.reciprocal(inv[:], s_b[:])
                for hf in range(2):
                    nc.vector.tensor_scalar_mul(
                        ynorm[b][:, ds(hf * 512, 512)], yps[:, hf, :], inv[:])

                # ---------------- per-batch epilogue (inline) ------------
                ystage = ps_stage.tile([128, NCH, H], F32, tag="stage")
                for ch in range(NCH):
                    nc.tensor.transpose(ystage[:, ch, :],
                                        ynorm[b][:, ds(ch * 128, 128)],
                                        ident[0:H, 0:H])
                ynr = ptmp.tile([128, NCH, H], F32R, tag="ynr")
                nc.vector.tensor_copy(ynr[:], ystage[:])

                gh = [ps_acc.tile([H, 512], F32, tag="acc", name=f"gh{_}") for _ in range(2)]
                for ch in range(NCH):
                    for hf in range(2):
                        nc.tensor.matmul(gh[hf][:], ynr[:, ch, :],
                                         wvr[:, ch, ds(hf * 512, 512)],
                                         start=(ch == 0), stop=(ch == NCH - 1))
                gm = ptmp.tile([H, DIM], F32R, tag="gm", bufs=1)
                for hf in range(2):
                    nc.vector.tensor_mul(gm[:, ds(hf * 512, 512)], gh[hf][:],
                                         mask[:, ds(hf * 512, 512)])
                zh = [ps_acc.tile([1, 512], F32, tag="acc", name=f"zh{_}") for _ in range(2)]
                for hf in range(2):
                    nc.tensor.matmul(zh[hf][:], ones16[:],
                                     gm[:, ds(hf * 512, 512)],
                                     start=True, stop=True)
                z_sb = ptmp.tile([1, DIM], F32, tag="z", bufs=1)
                for hf in range(2):
                    nc.scalar.copy(out=z_sb[:, ds(hf * 512, 512)], in_=zh[hf][:])

                ztp = ps_a.tile([128, NCH], F32, tag="a")
                for ch in range(NCH):
                    nc.tensor.transpose(ztp[:, ch:ch + 1],
                                        z_sb[0:1, ds(ch * 128, 128)],
                                        ident[0:1, 0:1])
                zt = ptmp.tile([128, NCH], F32R, tag="zt")
                nc.vector.tensor_copy(zt[:], ztp[:])

                oh = [ps_acc.tile([1, 512], F32, tag="acc", name=f"oh{_}") for _ in range(2)]
                for ch in range(NCH):
                    for hf in range(2):
                        nc.tensor.matmul(oh[hf][:], zt[:, ch:ch + 1],
                                         wpr[:, ch, ds(hf * 512, 512)],
                                         start=(ch == 0), stop=(ch == NCH - 1))
                ob = ptmp.tile([1, DIM], F32, tag="ob")
                for hf in range(2):
                    nc.vector.tensor_add(ob[:, ds(hf * 512, 512)], oh[hf][:],
                                         bp_sb[:, ds(hf * 512, 512)])
                nc.sync.dma_start(T["out"][b:b + 1, :], ob[:])


def _build():
    if "nc" in _CACHE:
        return _CACHE["nc"]
    nc = bacc.Bacc("TRN2", target_bir_lowering=False, debug=False,
                   num_devices=NCORES)
    T = {}
    T["x"] = nc.dram_tensor("x", [BPC, N, DIM], F32, kind="ExternalInput").ap()
    T["x0t"] = nc.dram_tensor("x0t", [128, NCH, BPC], F32,
                              kind="ExternalInput").ap()
    for w in ("wq", "wkt", "wv", "wp"):
        T[w] = nc.dram_tensor(w, [DIM, DIM], F32, kind="ExternalInput").ap()
    T["bp"] = nc.dram_tensor("bp", [1, DIM], F32, kind="ExternalInput").ap()
    T["lt"] = nc.dram_tensor("lt", [128, NCH, 128], F32,
                             kind="ExternalInput").ap()
    T["mask"] = nc.dram_tensor("mask", [H, DIM], F32, kind="ExternalInput").ap()
    T["out"] = nc.dram_tensor("out", [BPC, DIM], F32, kind="ExternalOutput").ap()

    with tile.TileContext(nc) as tc:
        _emit(tc, T)
    nc.compile()
    _CACHE["nc"] = nc
    return nc


# ------------------------------------------------------------------ host side
def _in_maps(x, Wq, Wk, Wv, Wp, bp):
    lt, mask = _host_tables()
    wkt = np.ascontiguousarray(Wk.T)
    bp1 = np.ascontiguousarray(bp.reshape(1, DIM))
    maps = []
    for c in range(NCORES):
        xs = np.ascontiguousarray(x[BPC * c:BPC * (c + 1)])
        x0 = xs[:, 0, :]                                     # (BPC, DIM)
        x0t = np.ascontiguousarray(
            x0.T.reshape(NCH, 128, BPC).transpose(1, 0, 2))  # (128, NCH, BPC)
        maps.append({"x": xs, "x0t": x0t, "wq": Wq, "wkt": wkt, "wv": Wv,
                     "wp": Wp, "bp": bp1, "lt": lt, "mask": mask})
    return maps


def run(x, Wq, Wk, Wv, Wp, bp, **kwargs):
    nc = _build()
    maps = _in_maps(x, Wq, Wk, Wv, Wp, bp)
    res = run_bass_kernel_spmd(nc, maps, core_ids=list(range(NCORES)), **kwargs)
    out = np.stack([r["out"] for r in res.results])          # (8, BPC, DIM)
    return out.reshape(B, 1, DIM), res


def kernel(x, Wq, Wk, Wv, Wp, bp):
    x = np.ascontiguousarray(np.asarray(x), dtype=np.float32)
    Wq = np.ascontiguousarray(np.asarray(Wq), dtype=np.float32)
    Wk = np.ascontiguousarray(np.asarray(Wk), dtype=np.float32)
    Wv = np.ascontiguousarray(np.asarray(Wv), dtype=np.float32)
    Wp = np.ascontiguousarray(np.asarray(Wp), dtype=np.float32)
    bp = np.ascontiguousarray(np.asarray(bp), dtype=np.float32)
    out, _ = run(x, Wq, Wk, Wv, Wp, bp)
    return out
